# revision 30
# baseline (speedup 1.0000x reference)
"""GraphSAGE 3-layer GNN forward pass on 8 Trainium2 NeuronCores.

Sharding: nodes split by range across 8 cores (graph/data parallel),
then permuted into 12800 slots per core (100 dst tiles of 128) by a
greedy balancer so every (dst-tile, bucket) needs ~4 edge chunks.
Slot quarters (3200) define the 4 gather buckets of the replicated
message table: bucket k rows = [core0 q_k | core1 q_k | ...] so each
bucket is produced by its own AllGather, issued as soon as the z-phase
finishes that quarter (collective overlaps compute).

Per layer the message table z = h @ Wl is computed shard-wise in bf16
and AllGathered into 4 bucket tables [25600, 128] bf16 (cols 0:F
valid, 256B rows); each core aggregates the edges whose dst is in its
shard: dma_gather (4 SWDGE queues, 8-chunk sub-calls, single_packet)
pulls z[src] rows into SBUF and bf16xfp8 one-hot matmuls on the tensor
engine do the segment-sum into PSUM. The one-hot matrices are
graph-static and layer-invariant: precomputed on the host in fp8 and
streamed from DRAM. Mean-normalization (1/deg), the self term h @ Wr,
BatchNorm (stats AllReduced), ReLU and log_softmax run on
vector/scalar engines.
"""

import numpy as np
import ml_dtypes

# ---------------- problem constants (hardcoded per contract) ----------------
N = 100000
E = 1600000
FIN = 200
NCORES = 8
NPC = N // NCORES            # 12500 nodes per core
NT = 100                     # dst tiles of 128 slots per core
NPAD = NT * 128              # 12800 slots
SHARD = NPAD
QT = NPAD // 4               # 3200 slots per quarter
SUB0 = 13 * 128              # first sub-range of a quarter (1664 slots)
SUB1 = QT - SUB0             # second sub-range (1536 slots)
NBUCK = 4
BROWS = QT * NCORES          # 25600 rows per bucket table (< 32768, int16)
F1, F2, F3 = 64, 32, 17
EPS = 1e-5

# ---------------- tunables ----------------
NQ = 4                # SWDGE queues used for gather calls
NSQ = 4               # queues declared (allows NQ up to 4)
GSIZE = 8             # dst tiles per PSUM accumulation group
SUBC = 8              # chunks per gather sub-call (64 descs/engine packet cap)
GBUFS = 12            # gather buffers in flight
PBUFS = 4             # one-hot P stream buffers in flight
CAP = 512             # target edges per (tile, bucket)


def _wrap16(idx_flat):
    """dma_gather index layout: position i -> partition i%16, col i//16,
    replicated across the 8 q7 core pairs (128 partitions)."""
    n = idx_flat.shape[0]
    w = idx_flat.reshape(n // 16, 16).T.copy()
    return np.tile(w, (8, 1))


def _assign_slots(edge_index):
    """Per-core node->slot permutation. Quarters are balanced by total
    in-degree; tiles within a quarter are greedily packed so each
    (tile, bucket) stays under CAP edges. Pad slots stay at the end
    (slots NPC..NPAD)."""
    src = np.asarray(edge_index[0], dtype=np.int64)
    dst = np.asarray(edge_index[1], dtype=np.int64)
    indeg = np.bincount(dst, minlength=N)

    # --- quarter assignment (defines src buckets): place each node (in
    # descending in-degree order) into the non-full quarter with the
    # smallest accumulated in-degree ---
    quarter = np.zeros(N, np.int64)
    qcap_real = np.array([QT, QT, QT, QT - (NPAD - NPC)])  # pads -> quarter 3
    for c in range(NCORES):
        deg = indeg[c * NPC:(c + 1) * NPC]
        order = np.argsort(-deg, kind="stable")
        fill = np.zeros(4, np.int64)
        load = np.zeros(4, np.int64)
        qa = np.zeros(NPC, np.int64)
        for n in order:
            ok = np.where(fill < qcap_real)[0]
            qi = ok[np.argmin(load[ok])]
            qa[n] = qi
            fill[qi] += 1
            load[qi] += deg[n]
        quarter[c * NPC:(c + 1) * NPC] = qa

    bucket_e = quarter[src]          # edge bucket = src quarter

    # --- per-core tile packing within quarters: snake-deal for even node
    # counts, then swap repair against 2 uncapped dump tiles per quarter
    # so non-dump (tile, bucket) loads stay under CAP ---
    perm = np.zeros((NCORES, NPAD), np.int64)   # node (padded) -> slot
    for c in range(NCORES):
        m = (dst >= c * NPC) & (dst < (c + 1) * NPC)
        dloc = dst[m] - c * NPC
        db = np.zeros((NPC, NBUCK), np.int64)
        np.add.at(db, (dloc, bucket_e[m]), 1)
        deg = db.sum(axis=1)
        qa = quarter[c * NPC:(c + 1) * NPC]
        pslot = np.zeros(NPC, np.int64)
        for q in range(4):
            nodes = np.where(qa == q)[0]
            nodes = nodes[np.argsort(-deg[nodes], kind="stable")]
            t0 = q * 25
            ntile = 25
            ncap = np.full(ntile, 128, np.int64)
            dumps = [20, 21] if q == 3 else [23, 24]
            if q == 3:
                # slots NPC.. are pads: tile 97 keeps 84 real, 98/99 none
                ncap[-3] = 84
                ncap[-2] = 0
                ncap[-1] = 0
            capped = np.ones(ntile, bool)
            capped[dumps] = False
            members = [[] for _ in range(ntile)]
            fill = np.zeros((ntile, NBUCK), np.int64)
            used = np.zeros(ntile, np.int64)
            seq = list(range(ntile)) + list(range(ntile - 1, -1, -1))
            si = 0
            for n in nodes:
                while used[seq[si % len(seq)]] >= ncap[seq[si % len(seq)]]:
                    si += 1
                t = seq[si % len(seq)]
                si += 1
                members[t].append(n)
                used[t] += 1
                fill[t] += db[n]
            for _ in range(20000):
                viol = np.where(capped[:, None], fill - CAP, 0)
                vtot = np.maximum(viol, 0).sum(axis=1)
                t = int(np.argmax(vtot))
                if vtot[t] <= 0:
                    break
                mu = members[t]
                du = db[mu]
                old = vtot[t]
                best = None
                for t2 in dumps:
                    mv = members[t2]
                    dv = db[mv]
                    w = np.maximum(fill[t] - CAP, 0)[None, :]
                    uscore = (du * (w > 0)).sum(axis=1)
                    for ui in np.argsort(-uscore)[:12]:
                        nf_part = fill[t] - du[ui]
                        nvi = np.maximum(nf_part[None, :] + dv - CAP, 0).sum(axis=1)
                        vi = int(np.argmin(nvi))
                        if nvi[vi] < old:
                            cand = (int(nvi[vi]), t2, int(ui), vi)
                            if best is None or cand[0] < best[0]:
                                best = cand
                    if best and best[0] == 0:
                        break
                if best is None:
                    break
                _, t2, ui, vi = best
                u = members[t][ui]
                v = members[t2][vi]
                members[t][ui] = v
                members[t2][vi] = u
                fill[t2] += db[u] - db[v]
                fill[t] += db[v] - db[u]
            for t in range(ntile):
                for k, n in enumerate(members[t]):
                    pslot[n] = (t0 + t) * 128 + k
        perm[c, :NPC] = pslot
        # pads fill the remaining (tail) slots in order
        taken = np.zeros(NPAD, bool)
        taken[pslot] = True
        perm[c, NPC:] = np.where(~taken)[0]
    return perm, bucket_e


def _preprocess(edge_index):
    src = np.asarray(edge_index[0], dtype=np.int64)
    dst = np.asarray(edge_index[1], dtype=np.int64)
    perm, bucket_e = _assign_slots(edge_index)

    src_core = src // NPC
    sslot = perm[src_core, src % NPC]
    bucket = sslot // QT
    # bucket table layout: rank-concatenated quarters [core0 q_k | ...].
    # Quarter 0 is further split in two sub-ranges (its AllGather gates the
    # first gathers of every layer, so its halves ship separately/earlier).
    r = sslot % QT
    rel = np.where(bucket != 0, src_core * QT + r,
                   np.where(r < SUB0, src_core * SUB0 + r,
                            NCORES * SUB0 + src_core * SUB1 + (r - SUB0)))

    dst_core = dst // NPC
    dslot = perm[dst_core, dst % NPC]
    tile_e = dslot >> 7
    dstrel_e = dslot & 127

    per_core = []
    needed = np.zeros((NCORES, NT, NBUCK), np.int64)
    for c in range(NCORES):
        m = dst_core == c
        key = tile_e[m] * NBUCK + bucket[m]
        order = np.argsort(key, kind="stable")
        cnts = np.bincount(key, minlength=NT * NBUCK).reshape(NT, NBUCK)
        per_core.append({
            "key": key[order],
            "rel": rel[m][order],
            "dstrel": dstrel_e[m][order],
            "cnt": np.bincount(dslot[m], minlength=NPAD),
            "cnts": cnts,
        })
        needed[c] = (cnts + 127) >> 7
    csched = np.maximum(needed.max(axis=0), 1)   # [NT, NBUCK]

    groups = [list(range(g, min(g + GSIZE, NT))) for g in range(0, NT, GSIZE)]
    chunk_start = np.zeros((NT, NBUCK), np.int64)
    calls = []  # (bucket, chunk_qstart, nchunks, group_index)
    q = 0
    for gi, g in enumerate(groups):
        for b in range(NBUCK):
            nch = 0
            for t in g:
                chunk_start[t, b] = q + nch
                nch += int(csched[t, b])
            calls.append((b, q, nch, gi))
            q += nch
    nchunk = q
    tile_of_chunk = np.zeros(nchunk, np.int64)
    for t in range(NT):
        for b in range(NBUCK):
            s = chunk_start[t, b]
            tile_of_chunk[s:s + csched[t, b]] = t

    idx_all = np.zeros((NCORES, 128, nchunk * 8), np.int16)
    P_all = np.zeros((NCORES, 128, nchunk * 128), ml_dtypes.float8_e4m3fn)
    rcnt_nm = np.zeros((NCORES, 128, NT), np.float32)
    rcnt_row = np.zeros((NCORES, NPAD), np.float32)
    vals = np.arange(128, dtype=np.int64)
    for c in range(NCORES):
        ck = per_core[c]
        seg_off = np.zeros(NT * NBUCK + 1, np.int64)
        seg_off[1:] = np.cumsum(ck["cnts"].reshape(-1))
        pos = np.arange(len(ck["key"])) - seg_off[ck["key"]]
        t_e = ck["key"] // NBUCK
        b_e = ck["key"] % NBUCK
        qg = chunk_start[t_e, b_e] + (pos >> 7)
        p = pos & 127
        idx_flat = np.zeros(nchunk * 128, np.int16)
        idx_flat[qg * 128 + p] = ck["rel"].astype(np.int16)
        idx_all[c] = _wrap16(idx_flat)
        dstrel = np.full((128, nchunk), -1, np.int64)
        dstrel[p, qg] = ck["dstrel"]
        P_all[c] = (dstrel[:, :, None] == vals[None, None, :]).astype(
            ml_dtypes.float8_e4m3fn).reshape(128, nchunk * 128)
        rc_pad = np.ones(NPAD, np.float32)
        rc_pad[:] = 1.0 / np.maximum(ck["cnt"], 1).astype(np.float32)
        rcnt_nm[c] = rc_pad.reshape(NT, 128).T
        rcnt_row[c] = rc_pad

    return {
        "csched": csched, "groups": groups, "calls": calls, "nchunk": nchunk,
        "chunk_start": chunk_start, "tile_of_chunk": tile_of_chunk,
        "idx_all": idx_all, "P_all": P_all, "perm": perm,
        "rcnt_nm": rcnt_nm, "rcnt_row": rcnt_row,
    }


def _build_program(pp):
    import concourse.bacc as bacc
    import concourse.tile as tile
    import concourse.mybir as mybir

    f32 = mybir.dt.float32
    bf16 = mybir.dt.bfloat16
    fp8 = mybir.dt.float8e4
    AX = mybir.AxisListType
    ALU = mybir.AluOpType
    ACT = mybir.ActivationFunctionType

    groups = pp["groups"]
    calls = pp["calls"]
    csched = pp["csched"]
    chunk_start = pp["chunk_start"]
    tile_of_chunk = pp["tile_of_chunk"]
    nchunk = pp["nchunk"]
    max_call_chunks = max(nc_ for (_, _, nc_, _) in calls)
    # (quarter, sub) -> z-phase group index after which it is fully written
    ag_after = {(0, 0): 1, (0, 1): 3, (1, 0): 6, (2, 0): 9,
                (3, 0): len(groups) - 1}

    nc = bacc.Bacc("TRN2", target_bir_lowering=False, debug=False,
                   num_devices=NCORES, num_swdge_queues=NSQ,
                   dynamic_dma_scratch_size=16384)

    # ---------------- I/O ----------------
    t_xT = nc.dram_tensor("xT", [FIN, NPAD], bf16, kind="ExternalInput")
    t_idx = nc.dram_tensor("gidx", [128, nchunk * 8], mybir.dt.int16, kind="ExternalInput")
    t_P = nc.dram_tensor("Pmat", [128, nchunk * 128], fp8, kind="ExternalInput")
    t_rcnt_nm = nc.dram_tensor("rcnt_nm", [128, NT], f32, kind="ExternalInput")
    t_rcnt_fm = nc.dram_tensor("rcnt_fm", [64, NPAD], f32, kind="ExternalInput")
    t_W1l = nc.dram_tensor("W1l", [FIN, F1], bf16, kind="ExternalInput")
    t_W1r = nc.dram_tensor("W1r", [FIN, F1], bf16, kind="ExternalInput")
    t_W2l = nc.dram_tensor("W2lp", [F1, 64], bf16, kind="ExternalInput")
    t_W2r = nc.dram_tensor("W2r", [F1, F2], bf16, kind="ExternalInput")
    t_W3l = nc.dram_tensor("W3lp", [F2, 64], bf16, kind="ExternalInput")
    t_W3r = nc.dram_tensor("W3r", [F2, F3], bf16, kind="ExternalInput")
    t_g1 = nc.dram_tensor("g1", [F1, 1], f32, kind="ExternalInput")
    t_be1 = nc.dram_tensor("be1", [F1, 1], f32, kind="ExternalInput")
    t_g2 = nc.dram_tensor("g2", [F2, 1], f32, kind="ExternalInput")
    t_be2 = nc.dram_tensor("be2", [F2, 1], f32, kind="ExternalInput")
    t_b3 = nc.dram_tensor("b3rep", [128, F3], f32, kind="ExternalInput")
    t_out = nc.dram_tensor("out", [NPAD, F3], f32, kind="ExternalOutput")

    shards = [nc.dram_tensor(f"shard{l}", [SHARD, 128], bf16, kind="Internal")
              for l in (1, 2, 3)]
    zbuck = [[nc.dram_tensor(f"zb{l}_{k}", [BROWS, 128], bf16, kind="Internal",
                             addr_space="Shared") for k in range(4)]
             for l in (1, 2, 3)]
    zrT1_d = nc.dram_tensor("zrT1", [64, NPAD], f32, kind="Internal")
    zrT2_d = nc.dram_tensor("zrT2", [F2, NPAD], f32, kind="Internal")
    zr3_d = nc.dram_tensor("zr3", [NPAD, F3], f32, kind="Internal")
    bn_in1 = nc.dram_tensor("bn_in1", [F1, 2], f32, kind="Internal")
    bn_out1 = nc.dram_tensor("bn_out1", [F1, 2], f32, kind="Internal", addr_space="Shared")
    bn_in2 = nc.dram_tensor("bn_in2", [F2, 2], f32, kind="Internal")
    bn_out2 = nc.dram_tensor("bn_out2", [F2, 2], f32, kind="Internal", addr_space="Shared")

    RG = [list(range(NCORES))]
    GW = GSIZE * 128

    with tile.TileContext(nc) as tc:
        with tc.tile_pool(name="const", bufs=1) as constp, \
             tc.tile_pool(name="wpool", bufs=1) as wpool, \
             tc.tile_pool(name="stage", bufs=2) as stagep, \
             tc.tile_pool(name="sm3", bufs=3) as sm3p, \
             tc.tile_pool(name="slab", bufs=2) as slabp, \
             tc.tile_pool(name="gbuf", bufs=GBUFS) as gbufp, \
             tc.tile_pool(name="pbuf", bufs=PBUFS) as pbufp, \
             tc.tile_pool(name="zpsum", bufs=2, space="PSUM") as zpsum, \
             tc.tile_pool(name="spsum", bufs=2, space="PSUM") as spsum, \
             tc.tile_pool(name="s3psum", bufs=2, space="PSUM") as s3psum, \
             tc.tile_pool(name="small", bufs=1) as smallp:

            # ---- constants
            idx_t = constp.tile([128, nchunk * 8], mybir.dt.int16)
            nc.sync.dma_start(idx_t[:], t_idx.ap())
            rcnt_nm_t = constp.tile([128, NT], f32)
            nc.sync.dma_start(rcnt_nm_t[:], t_rcnt_nm.ap())
            b3rep = constp.tile([128, F3], f32)
            nc.sync.dma_start(b3rep[:], t_b3.ap())
            hb1 = constp.tile([64, NPAD], bf16, tag="hb1")
            hb2 = constp.tile([64, NPAD], bf16, tag="hb2")

            def wload(name, tt, shape, dt=bf16):
                w = wpool.tile(shape, dt, tag=name)
                nc.sync.dma_start(w[:], tt)
                return w

            W1l_a = wload("w1la", t_W1l.ap()[:128], [128, F1])
            W1l_b = wload("w1lb", t_W1l.ap()[128:], [72, F1])
            W1r_a = wload("w1ra", t_W1r.ap()[:128], [128, F1])
            W1r_b = wload("w1rb", t_W1r.ap()[128:], [72, F1])
            W2l_t = wload("w2l", t_W2l.ap(), [F1, 64])
            W2r_t = wload("w2r", t_W2r.ap(), [F1, F2])
            W3l_t = wload("w3l", t_W3l.ap(), [F2, 64])
            W3r_t = wload("w3r", t_W3r.ap(), [F2, F3])
            g1_t = wload("g1", t_g1.ap(), [F1, 1], f32)
            be1_t = wload("be1", t_be1.ap(), [F1, 1], f32)
            g2_t = wload("g2", t_g2.ap(), [F2, 1], f32)
            be2_t = wload("be2", t_be2.ap(), [F2, 1], f32)

            def ag_emit(layer_i, k, sub):
                if k == 0:
                    s0 = 0 if sub == 0 else SUB0
                    sw = SUB0 if sub == 0 else SUB1
                    o0 = 0 if sub == 0 else NCORES * SUB0
                else:
                    s0 = k * QT
                    sw = QT
                    o0 = 0
                with nc.named_scope(f"AG{layer_i}_{k}_{sub}"):
                    nc.gpsimd.collective_compute(
                        "AllGather", ALU.bypass, replica_groups=RG,
                        ins=[shards[layer_i - 1].ap()[s0:s0 + sw]],
                        outs=[zbuck[layer_i - 1][k].ap()[o0:o0 + NCORES * sw]])

            # ================= layer-1 z phase =================
            with nc.named_scope("L1z"):
                for gi, g in enumerate(groups):
                    gw = len(g) * 128
                    c0 = g[0] * 128
                    xa = slabp.tile([128, GW], bf16, tag="xa")
                    xb = slabp.tile([72, GW], bf16, tag="xb")
                    nc.sync.dma_start(xa[:, :gw], t_xT.ap()[:128, c0:c0 + gw])
                    nc.scalar.dma_start(xb[:, :gw], t_xT.ap()[128:, c0:c0 + gw])
                    zr_sl = stagep.tile([64, GW], f32, tag="zrslab")
                    for ti, t in enumerate(g):
                        xs_a = xa[:, ti * 128:(ti + 1) * 128]
                        xs_b = xb[:, ti * 128:(ti + 1) * 128]
                        pz = zpsum.tile([128, 128], f32, tag="zps")
                        nc.tensor.matmul(pz[:, :F1], xs_a, W1l_a[:], start=True, stop=False)
                        nc.tensor.matmul(pz[:, :F1], xs_b, W1l_b[:], start=False, stop=True)
                        zs = sm3p.tile([128, 64], bf16, tag="zstage")
                        nc.vector.tensor_copy(zs[:], pz[:, :F1])
                        nc.sync.dma_start(shards[0].ap()[t * 128:(t + 1) * 128, 0:64], zs[:])
                        pr = zpsum.tile([128, 128], f32, tag="zps")
                        nc.tensor.matmul(pr[:F1, :], W1r_a[:], xs_a, start=True, stop=False)
                        nc.tensor.matmul(pr[:F1, :], W1r_b[:], xs_b, start=False, stop=True)
                        nc.scalar.copy(zr_sl[:, ti * 128:(ti + 1) * 128], pr[:F1, :])
                    nc.sync.dma_start(zrT1_d.ap()[:, c0:c0 + gw], zr_sl[:, :gw])
                    for (k, sub), gg in ag_after.items():
                        if gg == gi:
                            ag_emit(1, k, sub)

            # ========== generic gather/aggregate ==========
            subq = [0]
            callmap = {(gi, b): (qs, nch) for (b, qs, nch, gi) in calls}

            def agg_layer(zb, Fw, fm, zr_src, h_sink, scope, final_cb=None):
                # iterate bucket-major within blocks of ILG groups: when the
                # bucket-b AllGather lands, ILG groups' worth of bucket-b
                # gathers are ready, so the Pool queue never head-of-line
                # blocks on the next collective for long.
                seq = []
                for gi in range(len(groups)):
                    for b in range(NBUCK):
                        seq.append((b,) + callmap[(gi, b)] + (gi,))
                stat_parts = []
                with nc.named_scope(scope):
                    psums = {}
                    for ci, (b, qs, nch, gi) in enumerate(seq):
                        g = groups[gi]
                        gw = len(g) * 128
                        c0 = g[0] * 128
                        if b == 0:
                            if fm:
                                psums[gi] = spsum.tile([Fw, GW], f32, tag="sacc",
                                                       name=f"sacc_{gi}")
                            else:
                                psums[gi] = s3psum.tile([128, GSIZE * F3], f32,
                                                        tag="sacc3",
                                                        name=f"sacc3_{gi}")
                        ps = psums[gi]
                        Pt = pbufp.tile([128, max_call_chunks * 128], fp8, tag="P")
                        nc.scalar.dma_start(
                            Pt[:, :nch * 128], t_P.ap()[:, qs * 128:(qs + nch) * 128])
                        for s0 in range(0, nch, SUBC):
                            sn = min(SUBC, nch - s0)
                            gb = gbufp.tile([128, SUBC, 128], bf16, tag="gb")
                            nc.gpsimd.dma_gather(
                                out_ap=gb[:, :sn, :],
                                in_ap=zb[b].ap(),
                                idxs_ap=idx_t[:, (qs + s0) * 8:(qs + s0 + sn) * 8],
                                num_idxs=sn * 128, num_idxs_reg=sn * 128,
                                elem_size=128, single_packet=True,
                                queue_num=subq[0] % NQ)
                            subq[0] += 1
                            for j0 in range(sn):
                                j = s0 + j0
                                qq = qs + j
                                t = int(tile_of_chunk[qq])
                                ti = t - g[0]
                                if fm:
                                    # PSUM accumulation groups are per 2KB bank
                                    # (= 4 tiles of 128 fp32 cols): start/stop
                                    # only on the bank's first/last matmul.
                                    bft = g[0] + (ti // 4) * 4
                                    blt = min(bft + 3, g[-1])
                                else:
                                    bft, blt = g[0], g[-1]
                                first = (b == 0) and (t == bft) and (qq == chunk_start[bft, 0])
                                last = (b == NBUCK - 1) and (t == blt) and \
                                    (qq == chunk_start[blt, NBUCK - 1] + csched[blt, NBUCK - 1] - 1)
                                if fm:
                                    nc.tensor.matmul(
                                        ps[:, ti * 128:(ti + 1) * 128],
                                        gb[:, j0, 0:Fw],
                                        Pt[:, j * 128:(j + 1) * 128],
                                        start=first, stop=last, skip_group_check=True)
                                else:
                                    nc.tensor.matmul(
                                        ps[:, ti * F3:(ti + 1) * F3],
                                        Pt[:, j * 128:(j + 1) * 128],
                                        gb[:, j0, 0:F3],
                                        start=first, stop=last, skip_group_check=True)
                        if b == NBUCK - 1:
                            if fm:
                                rc_sl = slabp.tile([64, GW], f32, tag="rcsl")
                                nc.sync.dma_start(rc_sl[:Fw, :gw], t_rcnt_fm.ap()[:Fw, c0:c0 + gw])
                                zr_sl2 = slabp.tile([64, GW], f32, tag="zrsl2")
                                nc.sync.dma_start(zr_sl2[:Fw, :gw], zr_src[:, c0:c0 + gw])
                                hsl = stagep.tile([64, GW], f32, tag="hsl")
                                nc.vector.tensor_mul(hsl[:Fw, :gw], ps[:, :gw], rc_sl[:Fw, :gw])
                                nc.vector.tensor_add(hsl[:Fw, :gw], hsl[:Fw, :gw], zr_sl2[:Fw, :gw])
                                s_p = smallp.tile([Fw, 2], f32, tag=f"stat_{scope}_{gi}")
                                nc.vector.tensor_reduce(s_p[:, 0:1], hsl[:Fw, :gw],
                                                        axis=AX.X, op=ALU.add)
                                sq_scr = stagep.tile([64, GW], f32, tag="sqscr")
                                nc.scalar.activation(sq_scr[:Fw, :gw], hsl[:Fw, :gw],
                                                     ACT.Square, accum_out=s_p[:, 1:2])
                                stat_parts.append(s_p)
                                nc.vector.tensor_copy(h_sink[:Fw, c0:c0 + gw], hsl[:Fw, :gw])
                            else:
                                final_cb(ps, g, gi)
                return stat_parts

            def bn_finalize(stat_parts, Fw, bn_in, bn_out, g_t, be_t, scope):
                with nc.named_scope(scope):
                    np_ = len(stat_parts)
                    stk = smallp.tile([Fw, 2 * np_], f32, tag=f"stk_{scope}")
                    for i, s_p in enumerate(stat_parts):
                        nc.vector.tensor_copy(stk[:, 2 * i:2 * i + 2], s_p[:])
                    tot = smallp.tile([Fw, 2], f32, tag=f"tot_{scope}")
                    v = stk[:].rearrange("f (i two) -> f two i", two=2)
                    nc.vector.tensor_reduce(tot[:, 0:1], v[:, 0:1, :], axis=AX.X, op=ALU.add)
                    nc.vector.tensor_reduce(tot[:, 1:2], v[:, 1:2, :], axis=AX.X, op=ALU.add)
                    nc.sync.dma_start(bn_in.ap(), tot[:])
                    nc.gpsimd.collective_compute(
                        "AllReduce", ALU.add, replica_groups=RG,
                        ins=[bn_in.ap()], outs=[bn_out.ap()])
                    red = smallp.tile([Fw, 2], f32, tag=f"red_{scope}")
                    nc.sync.dma_start(red[:], bn_out.ap())
                    mean = smallp.tile([Fw, 1], f32, tag=f"mean_{scope}")
                    nc.vector.tensor_scalar_mul(mean[:], red[:, 0:1], 1.0 / N)
                    ex2 = smallp.tile([Fw, 1], f32, tag=f"ex2_{scope}")
                    nc.vector.tensor_scalar_mul(ex2[:], red[:, 1:2], 1.0 / N)
                    var = smallp.tile([Fw, 1], f32, tag=f"var_{scope}")
                    nc.vector.tensor_mul(var[:], mean[:], mean[:])
                    nc.vector.tensor_sub(var[:], ex2[:], var[:])
                    nc.vector.tensor_scalar_add(var[:], var[:], EPS)
                    std = smallp.tile([Fw, 1], f32, tag=f"std_{scope}")
                    nc.scalar.sqrt(std[:], var[:])
                    rstd = smallp.tile([Fw, 1], f32, tag=f"rstd_{scope}")
                    nc.vector.reciprocal(rstd[:], std[:])
                    scal = smallp.tile([Fw, 1], f32, tag=f"scal_{scope}")
                    nc.vector.tensor_mul(scal[:], g_t[:], rstd[:])
                    shift = smallp.tile([Fw, 1], f32, tag=f"shift_{scope}")
                    nc.vector.tensor_mul(shift[:], mean[:], scal[:])
                    nc.vector.tensor_sub(shift[:], be_t[:], shift[:])
                    return scal, shift

            stats1 = agg_layer(zbuck[0], F1, True, zrT1_d.ap(), hb1, "L1agg")
            scal1, shift1 = bn_finalize(stats1, F1, bn_in1, bn_out1, g1_t, be1_t, "BN1")

            # ================= layer-2 z phase =================
            with nc.named_scope("L2z"):
                for gi, g in enumerate(groups):
                    gw = len(g) * 128
                    c0 = g[0] * 128
                    hb = slabp.tile([64, GW], bf16, tag="hb")
                    nc.scalar.activation(hb[:F1, :gw], hb1[:F1, c0:c0 + gw], ACT.Relu,
                                         bias=shift1[:], scale=scal1[:])
                    if g[-1] == NT - 1:
                        nc.vector.memzero(hb[:F1, NPC - c0:gw])
                    zr_sl = stagep.tile([64, GW], f32, tag="zrslab")
                    for ti, t in enumerate(g):
                        hst = hb[:F1, ti * 128:(ti + 1) * 128]
                        pz = zpsum.tile([128, 128], f32, tag="zps")
                        nc.tensor.matmul(pz[:, :64], hst, W2l_t[:], start=True, stop=True)
                        zs = sm3p.tile([128, 64], bf16, tag="zstage")
                        nc.vector.tensor_copy(zs[:], pz[:, :64])
                        nc.sync.dma_start(shards[1].ap()[t * 128:(t + 1) * 128, 0:64], zs[:])
                        pr = zpsum.tile([128, 128], f32, tag="zps")
                        nc.tensor.matmul(pr[:F2, :], W2r_t[:], hst, start=True, stop=True)
                        nc.scalar.copy(zr_sl[:F2, ti * 128:(ti + 1) * 128], pr[:F2, :])
                    nc.sync.dma_start(zrT2_d.ap()[:, c0:c0 + gw], zr_sl[:F2, :gw])
                    for (k, sub), gg in ag_after.items():
                        if gg == gi:
                            ag_emit(2, k, sub)

            stats2 = agg_layer(zbuck[1], F2, True, zrT2_d.ap(), hb2, "L2agg")
            scal2, shift2 = bn_finalize(stats2, F2, bn_in2, bn_out2, g2_t, be2_t, "BN2")

            # ================= layer-3 z phase =================
            with nc.named_scope("L3z"):
                for gi, g in enumerate(groups):
                    gw = len(g) * 128
                    c0 = g[0] * 128
                    hb = slabp.tile([64, GW], bf16, tag="hb")
                    nc.scalar.activation(hb[:F2, :gw], hb2[:F2, c0:c0 + gw], ACT.Relu,
                                         bias=shift2[:], scale=scal2[:])
                    if g[-1] == NT - 1:
                        nc.vector.memzero(hb[:F2, NPC - c0:gw])
                    for ti, t in enumerate(g):
                        hst = hb[:F2, ti * 128:(ti + 1) * 128]
                        pz = zpsum.tile([128, 128], f32, tag="zps")
                        nc.tensor.matmul(pz[:, :64], hst, W3l_t[:], start=True, stop=True)
                        zs = sm3p.tile([128, 64], bf16, tag="zstage")
                        nc.vector.tensor_copy(zs[:], pz[:, :64])
                        nc.sync.dma_start(shards[2].ap()[t * 128:(t + 1) * 128, 0:64], zs[:])
                        pr = zpsum.tile([128, 128], f32, tag="zps")
                        nc.tensor.matmul(pr[:, :F3], hst, W3r_t[:], start=True, stop=True)
                        zs3 = sm3p.tile([128, F3], f32, tag="z3stage")
                        nc.scalar.copy(zs3[:], pr[:, :F3])
                        nc.sync.dma_start(zr3_d.ap()[t * 128:(t + 1) * 128], zs3[:])
                    for (k, sub), gg in ag_after.items():
                        if gg == gi:
                            ag_emit(3, k, sub)

            # layer-3 finalize: per-group mean/self/bias + log_softmax.
            # Exp runs per group; Ln is one batched op (no ACT table churn).
            h3all = constp.tile([128, NT * F3], f32, tag="h3all")
            seall = constp.tile([128, NT], f32, tag="seall")
            lsall = constp.tile([128, NT], f32, tag="lsall")
            l3_groups = []

            def l3_group(ps, g, gi):
                ng = len(g)
                g0 = g[0]
                h3 = h3all[:, g0 * F3:(g0 + ng) * F3]
                h3v = h3.rearrange("p (t f) -> p t f", f=F3)
                zr_sl3 = slabp.tile([128, GSIZE * F3], f32, tag="zrsl3")
                nc.sync.dma_start(
                    zr_sl3[:, :ng * F3].rearrange("p (t f) -> p t f", f=F3),
                    zr3_d.ap()[g0 * 128:(g0 + ng) * 128].rearrange(
                        "(t p) f -> p t f", p=128))
                nc.vector.tensor_tensor(
                    out=h3v,
                    in0=ps[:, :ng * F3].rearrange("p (t f) -> p t f", f=F3),
                    in1=rcnt_nm_t[:, g0:g0 + ng].rearrange(
                        "p (t o) -> p t o", o=1).to_broadcast([128, ng, F3]),
                    op=ALU.mult)
                nc.vector.tensor_add(h3, h3, zr_sl3[:, :ng * F3])
                nc.vector.tensor_tensor(
                    out=h3v, in0=h3v,
                    in1=b3rep[:].rearrange("p (o f) -> p o f", o=1)
                        .to_broadcast([128, ng, F3]),
                    op=ALU.add)
                mx = sm3p.tile([128, GSIZE], f32, tag="mx")
                nc.vector.tensor_reduce(
                    mx[:, :ng].rearrange("p (t o) -> p t o", o=1),
                    h3v, axis=AX.X, op=ALU.max)
                nc.vector.tensor_tensor(
                    out=h3v, in0=h3v,
                    in1=mx[:, :ng].rearrange("p (t o) -> p t o", o=1)
                        .to_broadcast([128, ng, F3]),
                    op=ALU.subtract)
                ex = sm3p.tile([128, GSIZE * F3], f32, tag="ex")
                nc.scalar.activation(ex[:, :ng * F3], h3, ACT.Exp)
                nc.vector.tensor_reduce(
                    seall[:, g0:g0 + ng].rearrange("p (t o) -> p t o", o=1),
                    ex[:, :ng * F3].rearrange("p (t f) -> p t f", f=F3),
                    axis=AX.X, op=ALU.add)
                l3_groups.append((g, gi))
                if gi == len(groups) - 2:
                    nc.scalar.activation(lsall[:, :g0 + ng], seall[:, :g0 + ng],
                                         ACT.Ln)

            agg_layer(zbuck[2], F3, False, zr3_d, None, "L3agg", final_cb=l3_group)

            with nc.named_scope("L3fin"):
                glast = groups[-1]
                nc.scalar.activation(lsall[:, glast[0]:], seall[:, glast[0]:],
                                     ACT.Ln)
                for g, gi in l3_groups:
                    ng = len(g)
                    g0 = g[0]
                    h3 = h3all[:, g0 * F3:(g0 + ng) * F3]
                    h3v = h3.rearrange("p (t f) -> p t f", f=F3)
                    nc.vector.tensor_tensor(
                        out=h3v, in0=h3v,
                        in1=lsall[:, g0:g0 + ng].rearrange("p (t o) -> p t o", o=1)
                            .to_broadcast([128, ng, F3]),
                        op=ALU.subtract)
                    nc.sync.dma_start(
                        t_out.ap()[g0 * 128:(g0 + ng) * 128].rearrange(
                            "(t p) f -> p t f", p=128),
                        h3v)

    nc.compile()
    return nc


_PROG_CACHE = {}


def _in_maps(pp, inputs):
    bf = ml_dtypes.bfloat16
    x = np.asarray(inputs["x"], np.float32)
    W2lp = np.zeros((F1, 64), np.float32)
    W2lp[:, :F2] = np.asarray(inputs["W2l"], np.float32)
    W3lp = np.zeros((F2, 64), np.float32)
    W3lp[:, :F3] = np.asarray(inputs["W3l"], np.float32)
    b3rep = np.broadcast_to(np.asarray(inputs["b3"], np.float32)[None, :], (128, F3)).copy()
    common = {
        "W1l": np.asarray(inputs["W1l"], bf),
        "W1r": np.asarray(inputs["W1r"], bf),
        "W2lp": W2lp.astype(bf),
        "W2r": np.asarray(inputs["W2r"], bf),
        "W3lp": W3lp.astype(bf),
        "W3r": np.asarray(inputs["W3r"], bf),
        "g1": np.asarray(inputs["g1"], np.float32)[:, None].copy(),
        "be1": np.asarray(inputs["be1"], np.float32)[:, None].copy(),
        "g2": np.asarray(inputs["g2"], np.float32)[:, None].copy(),
        "be2": np.asarray(inputs["be2"], np.float32)[:, None].copy(),
        "b3rep": b3rep,
    }
    in_maps = []
    for c in range(NCORES):
        xT = np.zeros((FIN, NPAD), bf)
        xT[:, pp["perm"][c, :NPC]] = x[c * NPC:(c + 1) * NPC].T.astype(bf)
        m = dict(common)
        m["xT"] = xT
        m["gidx"] = pp["idx_all"][c]
        m["Pmat"] = pp["P_all"][c]
        m["rcnt_nm"] = pp["rcnt_nm"][c]
        m["rcnt_fm"] = np.broadcast_to(pp["rcnt_row"][c][None, :], (64, NPAD)).copy()
        in_maps.append(m)
    return in_maps


def kernel(**inputs):
    edge_index = np.asarray(inputs["edge_index"])
    pp = _preprocess(edge_index)
    key = (pp["nchunk"], pp["csched"].tobytes())
    if key not in _PROG_CACHE:
        _PROG_CACHE[key] = _build_program(pp)
    nc = _PROG_CACHE[key]
    in_maps = _in_maps(pp, inputs)
    from concourse.bass_utils import run_bass_kernel_spmd
    res = run_bass_kernel_spmd(nc, in_maps, core_ids=list(range(NCORES)))
    out = np.empty((N, F3), np.float32)
    for c in range(NCORES):
        out[c * NPC:(c + 1) * NPC] = res.results[c]["out"][pp["perm"][c, :NPC]]
    return out
